# revision 11
# baseline (speedup 1.0000x reference)
"""Trainium2 Bass kernel for temporal-window GNN mean aggregation, v6.

    out = x + scatter_mean(x[src] * mask, dst),
    mask = (edge_time <= seed_time[dst]) & (edge_time > seed_time[dst] - 100)

Destination-node sharding across 8 cores (no collectives).  Host work is
layout only; all reference arithmetic (mask compare, compaction ranks,
segment sums/counts, divide, residual) happens on device.

v6 adds DEVICE-SIDE MASK COMPACTION: only ~9.6% of edges survive the
temporal mask, so the kernel computes the mask, ranks survivors with a
strict-lower-triangular one-hot matmul (partition prefix) plus a ones-matmul
column-sum, compacts (src_lo, src_hi, dst-key) through rank-one-hot matmuls,
and gathers ONLY survivor rows: 12800 descriptors/core instead of 100352
(the SWDGE drain is ~2-3ns/descriptor, so this is the whole ballgame).

Structure per core:
  - 100 windows of 128 dst; bins = (window, bank) with B=ceil(maxbin/128)
    metadata columns of 128 slots each.
  - compaction bins = (window-PAIR, bank), CAP=64 survivor slots; compacted
    blocks = (window-QUAD, bank) stacking two pair-bins (ranks 0..63/64..127
    via a +64 rank base on odd pairs) -> 25 quads x 4 banks = 100 blocks of
    128 slots.
  - 4 gather calls (one per 25089-row src bank, int16 SWDGE index limit),
    3200 indices each; empty ranks fetch spread dummy rows (never a constant
    row: hot-row conflicts serialize the drain).
  - aggregation: per quad, per window, 4 bank-matmuls accumulate
    PSUM[dst, 0:129] += S^T @ G (S = one-hot of the compacted dst key,
    column 128 of G is a constant 1.0 feeding the count).
  - mean via ACT scale with 1/max(cnt,1), residual add on DVE, store.
"""

import math
import sys

import numpy as np

for _p in ("/opt/trn_rl_repo",):
    if _p not in sys.path:
        sys.path.insert(0, _p)

import concourse.bass as bass
import concourse.mybir as mybir
import concourse.tile as tile
from concourse import bacc
from concourse.bass_utils import run_bass_kernel_spmd

P = 128            # SBUF partitions == dst-window size
D = 128            # feature dim
NCORES = 8
W = 100            # dst windows per core
CW = 4             # windows per quad (= processing chunk)
NCHUNK = W // CW   # 25 quads
NODES_PC = W * P   # 12800 dst nodes per core
NPAD = NCORES * NODES_PC  # 102400
TIME_WINDOW = 100
CAP = 64           # survivor slots per (window-pair, bank) bin

NBANKS = 4
BANK = 25089
XROWS = NBANKS * BANK
NBLK = NCHUNK * NBANKS      # 100 compacted blocks per core
NIDXB = NCHUNK * P          # 3200 gather indices per bank
ICOLS = NIDXB // 16         # 200 idx-plane columns per bank

f32 = mybir.dt.float32
f16 = mybir.dt.float16
i32 = mybir.dt.int32
i16 = mybir.dt.int16
OP = mybir.AluOpType


def build_program(B: int):
    """B = metadata columns (128-slot groups) per (window, bank) bin."""
    C0 = W * NBANKS * B          # metadata cols per core
    QC = CW * NBANKS * B         # metadata cols per quad (16B)
    RPC = CW * P                 # dst rows per quad chunk (512)
    nc = bacc.Bacc(
        "TRN2", target_bir_lowering=False, debug=False, num_devices=NCORES,
        num_swdge_queues=4,
    )

    x16 = nc.dram_tensor("x16", [XROWS, 2 * D], f16, kind="ExternalInput")
    xs = nc.dram_tensor("xs", [NODES_PC, D], f32, kind="ExternalInput")
    etf = nc.dram_tensor("etf", [P, C0], f16, kind="ExternalInput")
    stf = nc.dram_tensor("stf", [P, C0], f16, kind="ExternalInput")
    fld = nc.dram_tensor("fld", [P, C0 * 3], f16, kind="ExternalInput")
    rbs = nc.dram_tensor("rbs", [P, C0], f16, kind="ExternalInput")
    spf = nc.dram_tensor("spf", [P, NBLK], f32, kind="ExternalInput")
    # internal scratch for the index-plane wrap bounce (host ignores it)
    cidd = nc.dram_tensor("cidd", [P, NBLK], i16, kind="ExternalOutput")
    out = nc.dram_tensor("out", [NODES_PC, D], f32, kind="ExternalOutput")

    with tile.TileContext(nc) as tc:
        with (
            tc.tile_pool(name="meta", bufs=1) as meta,
            tc.tile_pool(name="rw", bufs=2) as rw,
            tc.tile_pool(name="sq", bufs=2) as sq,
            tc.tile_pool(name="oc", bufs=2) as oc,
            tc.tile_pool(name="small", bufs=4) as small,
            tc.tile_pool(name="ps0", bufs=1, space="PSUM") as ps0,
            tc.tile_pool(name="psc", bufs=2, space="PSUM") as psc,
            tc.tile_pool(name="psa", bufs=2, space="PSUM") as psa,
        ):
            # ------------- phase 0: metadata, mask, survivor ranks -------
            et_t = meta.tile([P, C0], f16)
            st_t = meta.tile([P, C0], f16)
            fld_t = meta.tile([P, C0 * 3], f16)
            rbs_t = meta.tile([P, C0], f16)
            spf_t = meta.tile([P, NBLK], f32)
            nc.sync.dma_start(out=et_t[:], in_=etf[:])
            nc.sync.dma_start(out=st_t[:], in_=stf[:])
            nc.sync.dma_start(out=fld_t[:], in_=fld[:])
            nc.sync.dma_start(out=rbs_t[:], in_=rbs[:])
            nc.sync.dma_start(out=spf_t[:], in_=spf[:])

            # constants: row id per partition, col iota, strict-lower T,
            # rank iota (0..127), window-quad dst iota (1..512), ones column
            rid_i = meta.tile([P, P], i32)
            nc.gpsimd.iota(rid_i[:], pattern=[[0, P]], base=0,
                           channel_multiplier=1)
            cid_i = meta.tile([P, P], i32)
            nc.gpsimd.iota(cid_i[:], pattern=[[1, P]], base=0,
                           channel_multiplier=0)
            rid_f = meta.tile([P, P], f16)
            nc.vector.tensor_copy(out=rid_f[:], in_=rid_i[:])
            cid_f = meta.tile([P, P], f16)
            nc.vector.tensor_copy(out=cid_f[:], in_=cid_i[:])
            t_t = meta.tile([P, P], f16)
            nc.vector.tensor_tensor(out=t_t[:], in0=rid_f[:], in1=cid_f[:],
                                    op=OP.is_lt)
            iot128 = meta.tile([P, P], f16)
            nc.vector.tensor_copy(out=iot128[:], in_=cid_f[:])
            i512_i = meta.tile([P, CW * P], i32)
            nc.gpsimd.iota(i512_i[:], pattern=[[1, CW * P]], base=1,
                           channel_multiplier=0)
            i512_f = meta.tile([P, CW * P], f16)
            nc.vector.tensor_copy(out=i512_f[:], in_=i512_i[:])
            ones_t = meta.tile([P, 1], f16)
            nc.vector.memset(ones_t[:], 1.0)

            # temporal mask on the slot grid
            d_t = meta.tile([P, C0], f16)
            nc.vector.tensor_tensor(out=d_t[:], in0=st_t[:], in1=et_t[:],
                                    op=OP.subtract)
            m1_t = meta.tile([P, C0], f16)
            nc.vector.tensor_scalar(out=m1_t[:], in0=d_t[:], scalar1=0.0,
                                    scalar2=None, op0=OP.is_ge)
            m2_t = meta.tile([P, C0], f16)
            nc.vector.tensor_scalar(out=m2_t[:], in0=d_t[:],
                                    scalar1=float(TIME_WINDOW),
                                    scalar2=None, op0=OP.is_lt)
            m_t = meta.tile([P, C0], f16)
            nc.vector.tensor_tensor(out=m_t[:], in0=m1_t[:], in1=m2_t[:],
                                    op=OP.mult)

            # survivor ranks: exclusive partition prefix (T^T @ m) plus
            # per-bin column bases (ones^T @ m, broadcast, accumulated)
            HC = C0 // 2
            cp_t = meta.tile([P, C0], f16)
            cs_t = meta.tile([1, C0], f16)
            for h in range(2):
                sl = slice(h * HC, (h + 1) * HC)
                pp = ps0.tile([P, HC], f32, tag="pp")
                nc.tensor.matmul(out=pp[:], lhsT=t_t[:], rhs=m_t[:, sl],
                                 start=True, stop=True)
                nc.vector.tensor_copy(out=cp_t[:, sl], in_=pp[:])
                sp = ps0.tile([1, HC], f32, tag="sp")
                nc.tensor.matmul(out=sp[:], lhsT=ones_t[:], rhs=m_t[:, sl],
                                 start=True, stop=True)
                nc.vector.tensor_copy(out=cs_t[:, sl], in_=sp[:])
            csb_t = meta.tile([P, C0], f16)
            nc.gpsimd.partition_broadcast(out_ap=csb_t[:], in_ap=cs_t[:])

            # rank[p, bin, c] = cp[p, bin, c] + sum_{c'<c} colsum[bin, c']
            # where a bin = (window-PAIR, bank) spans BB = 2*B columns
            BB = 2 * B
            rk_t = meta.tile([P, C0], f16)
            rk3 = rk_t[:].rearrange("p (b c) -> p b c", c=BB)
            cp3 = cp_t[:].rearrange("p (b c) -> p b c", c=BB)
            cs3 = csb_t[:].rearrange("p (b c) -> p b c", c=BB)
            nc.vector.tensor_copy(out=rk3[:, :, 0], in_=cp3[:, :, 0])
            acc_t = meta.tile([P, C0 // BB], f16)
            nc.vector.tensor_copy(out=acc_t[:], in_=cs3[:, :, 0])
            for c in range(1, BB):
                nc.vector.tensor_tensor(out=rk3[:, :, c],
                                        in0=cp3[:, :, c], in1=acc_t[:],
                                        op=OP.add)
                if c < BB - 1:
                    nc.vector.tensor_tensor(out=acc_t[:], in0=acc_t[:],
                                            in1=cs3[:, :, c],
                                            op=OP.add)

            # pair-bin overflow (rank >= CAP) -> drop instead of corrupt
            ov_t = meta.tile([P, C0], f16)
            nc.vector.tensor_scalar(out=ov_t[:], in0=rk_t[:],
                                    scalar1=float(CAP), scalar2=300.0,
                                    op0=OP.is_ge, op1=OP.mult)
            # ranke = rank + rbase + 300*(1-m) + 300*ovf
            nm_t = meta.tile([P, C0], f16)
            nc.vector.tensor_scalar(out=nm_t[:], in0=m_t[:], scalar1=-300.0,
                                    scalar2=300.0, op0=OP.mult, op1=OP.add)
            rke_t = meta.tile([P, C0], f16)
            nc.vector.tensor_tensor(out=rke_t[:], in0=rk_t[:], in1=rbs_t[:],
                                    op=OP.add)
            nc.vector.tensor_tensor(out=rke_t[:], in0=rke_t[:], in1=nm_t[:],
                                    op=OP.add)
            nc.vector.tensor_tensor(out=rke_t[:], in0=rke_t[:], in1=ov_t[:],
                                    op=OP.add)

            # ------------- phase 1: compact fields via rank one-hots -----
            cdl_t = meta.tile([P, NBLK], f16)
            cid2_t = meta.tile([P, NBLK * 2], f32)
            for Q in range(NCHUNK):
                r_t = rw.tile([P, QC * P], f16, tag="r")
                nc.vector.tensor_tensor(
                    out=r_t[:].rearrange("p (oc r) -> p oc r", r=P),
                    in0=rke_t[:, Q * QC:(Q + 1) * QC]
                    .unsqueeze(2).to_broadcast([P, QC, P]),
                    in1=iot128[:].unsqueeze(1).to_broadcast([P, QC, P]),
                    op=OP.is_equal,
                )
                for j in range(NBANKS):
                    cm = psc.tile([P, 3], f32, tag="cm")
                    nmm = CW * B
                    k = 0
                    for pl in range(2):
                        for w2 in range(2):
                            for c in range(B):
                                ocl = ((pl * NBANKS + j) * 2 + w2) * B + c
                                gcol = Q * QC + ocl
                                nc.tensor.matmul(
                                    out=cm[:],
                                    lhsT=r_t[:, ocl * P:(ocl + 1) * P],
                                    rhs=fld_t[:, gcol * 3:gcol * 3 + 3],
                                    start=(k == 0),
                                    stop=(k == nmm - 1),
                                )
                                k += 1
                    bcol = Q * NBANKS + j
                    nc.vector.tensor_copy(out=cdl_t[:, bcol:bcol + 1],
                                          in_=cm[:, 2:3])
                    nc.vector.tensor_copy(
                        out=cid2_t[:, bcol * 2:bcol * 2 + 2], in_=cm[:, 0:2])

            # recombine src index: idx = (lo+1) + 128*hi - 1; empty ranks
            # (idx == -1) get spread dummy rows (hot-row hazard otherwise)
            ci3 = cid2_t[:].rearrange("p (b two) -> p b two", two=2)
            t1_t = meta.tile([P, NBLK], f32)
            nc.vector.tensor_scalar(out=t1_t[:], in0=ci3[:, :, 1],
                                    scalar1=128.0, scalar2=None, op0=OP.mult)
            ixf_t = meta.tile([P, NBLK], f32)
            nc.vector.tensor_tensor(out=ixf_t[:], in0=t1_t[:],
                                    in1=ci3[:, :, 0], op=OP.add)
            nc.vector.tensor_scalar(out=ixf_t[:], in0=ixf_t[:], scalar1=1.0,
                                    scalar2=None, op0=OP.subtract)
            isn_t = meta.tile([P, NBLK], f32)
            nc.vector.tensor_scalar(out=isn_t[:], in0=ixf_t[:], scalar1=0.0,
                                    scalar2=None, op0=OP.is_lt)
            df_t = meta.tile([P, NBLK], f32)
            nc.vector.tensor_tensor(out=df_t[:], in0=spf_t[:], in1=ixf_t[:],
                                    op=OP.subtract)
            nc.vector.tensor_tensor(out=df_t[:], in0=df_t[:], in1=isn_t[:],
                                    op=OP.mult)
            nc.vector.tensor_tensor(out=ixf_t[:], in0=ixf_t[:], in1=df_t[:],
                                    op=OP.add)
            ci_t = meta.tile([P, NBLK], i16)
            nc.vector.tensor_copy(out=ci_t[:], in_=ixf_t[:])

            # ------------- phase 2: wrap bounce via DRAM -----------------
            nc.sync.dma_start(out=cidd[:], in_=ci_t[:])
            ipl_t = meta.tile([P, NBANKS * ICOLS], i16)
            cid_w = cidd[:].rearrange("(pp b) (q j) -> b q pp j",
                                      b=16, j=NBANKS)
            for j in range(NBANKS):
                for a in range(8):
                    nc.sync.dma_start(
                        out=ipl_t[a * 16:(a + 1) * 16,
                                  j * ICOLS:(j + 1) * ICOLS]
                        .rearrange("b (q pp) -> b q pp", pp=8),
                        in_=cid_w[:, :, :, j],
                    )

            # ------------- phase 3: survivor gathers ---------------------
            gx = []
            for j in range(NBANKS):
                g = meta.tile([P, NCHUNK * 2 * D], f16, tag=f"gx{j}")
                gx.append(g)
            for j in range(NBANKS):
                nc.gpsimd.dma_gather(
                    out_ap=gx[j][:].rearrange("p (k c) -> p k c", c=2 * D),
                    in_ap=x16[j * BANK:, :],
                    idxs_ap=ipl_t[:, j * ICOLS:(j + 1) * ICOLS],
                    num_idxs=NIDXB,
                    num_idxs_reg=NIDXB,
                    elem_size=2 * D,
                    single_packet=False,
                    queue_num=j,
                )

            # ------------- phase 4: aggregate, mean, residual, store -----
            for Q in range(NCHUNK):
                s_t = sq.tile([P, NBANKS * CW * P], f16, tag="s")
                nc.vector.tensor_tensor(
                    out=s_t[:].rearrange("p (j wq d) -> p j wq d",
                                         wq=CW, d=P),
                    in0=i512_f[:].rearrange("p (wq d) -> p wq d", d=P)
                    .unsqueeze(1).to_broadcast([P, NBANKS, CW, P]),
                    in1=cdl_t[:, Q * NBANKS:(Q + 1) * NBANKS]
                    .unsqueeze(2).unsqueeze(3)
                    .to_broadcast([P, NBANKS, CW, P]),
                    op=OP.is_equal,
                )
                x_t = oc.tile([P, CW * D], f32, tag="x")
                nc.sync.dma_start(
                    out=x_t[:],
                    in_=xs[Q * RPC:(Q + 1) * RPC, :].rearrange(
                        "(p w) d -> p (w d)", p=P),
                )
                o_t = oc.tile([P, CW * D], f32, tag="o")
                for wq in range(CW):
                    ps = psa.tile([P, D + 1], f32, tag="ps")
                    for j in range(NBANKS):
                        nc.tensor.matmul(
                            out=ps[:],
                            lhsT=s_t[:, (j * CW + wq) * P:
                                     (j * CW + wq + 1) * P],
                            rhs=gx[j][:, Q * 2 * D:Q * 2 * D + D + 1],
                            start=(j == 0),
                            stop=(j == NBANKS - 1),
                        )
                    cnt_t = small.tile([P, 1], f32, tag="cnt")
                    nc.vector.tensor_scalar(out=cnt_t[:], in0=ps[:, D:D + 1],
                                            scalar1=1.0, scalar2=None,
                                            op0=OP.max)
                    rcp_t = small.tile([P, 1], f32, tag="rcp")
                    nc.vector.reciprocal(out=rcp_t[:], in_=cnt_t[:])
                    osl = o_t[:, wq * D:(wq + 1) * D]
                    nc.scalar.activation(
                        out=osl, in_=ps[:, 0:D],
                        func=mybir.ActivationFunctionType.Copy,
                        scale=rcp_t[:, 0:1],
                    )
                    nc.vector.tensor_tensor(
                        out=osl, in0=osl, in1=x_t[:, wq * D:(wq + 1) * D],
                        op=OP.add,
                    )
                nc.sync.dma_start(
                    out=out[Q * RPC:(Q + 1) * RPC, :].rearrange(
                        "(p w) d -> p (w d)", p=P),
                    in_=o_t[:],
                )

    nc.compile()
    return nc


_PROGRAM_CACHE: dict[int, object] = {}


def _get_program(B: int):
    if B not in _PROGRAM_CACHE:
        _PROGRAM_CACHE[B] = build_program(B)
    return _PROGRAM_CACHE[B]


def _perm_rows(a, nchunk, cw):
    return (
        a.reshape(nchunk, cw, P, -1).transpose(0, 2, 1, 3)
        .reshape(nchunk * cw * P, -1)
    )


def _unperm_rows(a, nchunk, cw):
    return (
        a.reshape(nchunk, P, cw, -1).transpose(0, 2, 1, 3)
        .reshape(nchunk * cw * P, -1)
    )


def _prep_inputs(x, edge_index, edge_time, seed_time):
    """Layout only: sort edges by (window, bank) into the metadata slot
    grid; ship mask operands + compaction fields.  No mask arithmetic."""
    x = np.asarray(x, dtype=np.float32)
    ei = np.asarray(edge_index)
    et = np.asarray(edge_time).astype(np.int64)
    st = np.asarray(seed_time).astype(np.int64)
    N = x.shape[0]
    E = ei.shape[1]
    assert N <= NPAD and N <= XROWS

    src = ei[0].astype(np.int64)
    dst = ei[1].astype(np.int64)

    win = dst // P                       # global window, 0..NCORES*W-1
    bank = src // BANK
    gid = win * NBANKS + bank
    order = np.argsort(gid, kind="stable")
    binc = np.bincount(gid, minlength=NCORES * W * NBANKS)
    B = max(1, int(math.ceil(binc.max() / P)))
    C0 = W * NBANKS * B

    offs = np.zeros(NCORES * W * NBANKS, dtype=np.int64)
    np.cumsum(binc[:-1], out=offs[1:])
    gs = gid[order]
    rank_pre = np.arange(E, dtype=np.int64) - offs[gs]
    win_s = gs // NBANKS
    bank_s = gs % NBANKS
    core_s = win_s // W
    winl = win_s % W
    c = rank_pre >> 7
    p = rank_pre & (P - 1)
    pair = winl // 2
    col = ((pair * NBANKS + bank_s) * 2 + (winl % 2)) * B + c

    et_a = np.zeros((NCORES, P, C0), dtype=np.float16)
    st_a = np.full((NCORES, P, C0), -2000.0, dtype=np.float16)
    fld_a = np.zeros((NCORES, P, C0 * 3), dtype=np.float16)
    et_a[core_s, p, col] = et[order].astype(np.float16)
    st_a[core_s, p, col] = st[dst[order]].astype(np.float16)
    srcl = src[order] - bank_s * BANK
    fld_a[core_s, p, col * 3 + 0] = (srcl % P + 1).astype(np.float16)
    fld_a[core_s, p, col * 3 + 1] = (srcl // P).astype(np.float16)
    fld_a[core_s, p, col * 3 + 2] = (
        (winl % CW) * P + (dst[order] % P) + 1
    ).astype(np.float16)

    # rank base: +64 for odd window-pairs within each quad (same all cores)
    pairid = np.arange(C0) // (NBANKS * 2 * B)    # pair per metadata col
    rb_row = (64.0 * (pairid % 2)).astype(np.float16)
    rbs_a = np.broadcast_to(rb_row, (P, C0)).copy()

    # spread dummy rows for empty compacted ranks
    sp = ((np.arange(P)[:, None] * NBLK + np.arange(NBLK)[None, :]) * 37
          ) % BANK
    spf_a = sp.astype(np.float32)

    x_pad = np.zeros((NPAD, D), dtype=np.float32)
    x_pad[:N] = x
    x16 = np.zeros((XROWS, 2 * D), dtype=np.float16)
    x16[:N, :D] = x.astype(np.float16)
    x16[:, D] = 1.0
    x_shards = x_pad.reshape(NCORES, NODES_PC, D)

    in_maps = [
        {
            "x16": x16,
            "xs": np.ascontiguousarray(_perm_rows(x_shards[cc], NCHUNK, CW)),
            "etf": et_a[cc],
            "stf": st_a[cc],
            "fld": fld_a[cc],
            "rbs": rbs_a,
            "spf": spf_a,
        }
        for cc in range(NCORES)
    ]
    return in_maps, B, N


def kernel(x, edge_index, edge_time, seed_time):
    in_maps, B, N = _prep_inputs(x, edge_index, edge_time, seed_time)
    nc = _get_program(B)
    res = run_bass_kernel_spmd(nc, in_maps, core_ids=list(range(NCORES)))
    out = np.concatenate(
        [_unperm_rows(res.results[c]["out"], NCHUNK, CW)
         for c in range(NCORES)],
        axis=0,
    )
    return np.ascontiguousarray(out[:N]).astype(np.float32)


# revision 16
# speedup vs baseline: 2.5015x; 2.5015x over previous
"""Trainium2 Bass kernel for temporal-window GNN mean aggregation.

    out = x + scatter_mean(x[src] * mask, dst),
    mask = (edge_time <= seed_time[dst]) & (edge_time > seed_time[dst] - 100)

Sharding: destination-node sharding across 8 cores (no collectives).
Host work is layout only: sort edges by (dst window, src bank), pad to a
uniform slot grid, build int16 gather-index planes (mask-independent), and
ship per-slot metadata (edge_time, seed_time[dst], dst%128).  All reference
arithmetic - the temporal mask compare, the masked segment sums / counts
(one-hot matmul on the PE array), the divide and the residual add - happens
on device.

Device per core (SPMD, one program):
  phase 0: wide DVE ops compute mask m per slot and fold it into the
           one-hot key dl_eff = (dst%128)+300-300*m (no iota match -> S=0).
  loop over chunks of CW windows (window = 128 consecutive dst):
    - 4x dma_gather on 4 SWDGE queues (one per 25089-row src bank, int16
      index limit) fetch 512-byte x16 rows = [128 features, 1.0, pad] for
      every slot; the ones column feeds the count accumulation
    - one batched DVE tensor_tensor builds the one-hot S = (iota == dl_eff)
    - PE per window: K matmuls accumulate PSUM[dst, 0:129] += S^T @ G
    - counts: max(cnt,1) + reciprocal batched per chunk, mean via ACT scale
    - residual: out rows += x rows via one accumulate-DMA per chunk
"""

import math
import sys

import numpy as np

for _p in ("/opt/trn_rl_repo",):
    if _p not in sys.path:
        sys.path.insert(0, _p)

import concourse.bass as bass
import concourse.mybir as mybir
import concourse.tile as tile
from concourse import bacc
from concourse.bass_utils import run_bass_kernel_spmd

P = 128            # SBUF partitions == dst-window size == edge-block size
D = 128            # feature dim
NCORES = 8
W = 98             # dst windows per core
CW = 7             # windows per processing chunk
NCHUNK = W // CW   # 14
NODES_PC = W * P   # 12544 dst nodes per core
NPAD = NCORES * NODES_PC  # 100352
TIME_WINDOW = 100

NBANKS = 4         # int16 gather-index banks over x16 rows
BANK = 25089       # rows per bank (<= 32768), NBANKS*BANK >= N
XROWS = NBANKS * BANK

f32 = mybir.dt.float32
f16 = mybir.dt.float16
i32 = mybir.dt.int32
i16 = mybir.dt.int16
OP = mybir.AluOpType


def build_program(B: int):
    """B = blocks per (window, bank); K = NBANKS*B blocks per window."""
    K = NBANKS * B
    C = W * K                    # metadata columns per core
    CBLK = CW * B                # blocks per (chunk, bank)
    NIDX = CBLK * P              # indices per gather call
    ICOLS = NIDX // 16           # idx columns per gather call
    RPC = CW * P                 # rows per chunk
    nc = bacc.Bacc(
        "TRN2", target_bir_lowering=False, debug=False, num_devices=NCORES,
        num_swdge_queues=4,
    )

    # x16 rows are 256 fp16 (512B): 128 features, a 1.0 ones column feeding
    # the count accumulation, then zero padding (dma_gather elem_size must
    # be a multiple of 256B).
    x16 = nc.dram_tensor("x16", [XROWS, 2 * D], f16, kind="ExternalInput")
    # xs/out use the host-permuted row order (chunk, partition, window):
    # row = chunk*CW*P + p*CW + wl, so chunk streams are fully contiguous.
    xs = nc.dram_tensor("xs", [NODES_PC, D], f32, kind="ExternalInput")
    idx16 = nc.dram_tensor(
        "idx16", [P, NCHUNK * NBANKS * ICOLS], i16, kind="ExternalInput"
    )
    etf = nc.dram_tensor("etf", [P, C], f16, kind="ExternalInput")
    stf = nc.dram_tensor("stf", [P, C], f16, kind="ExternalInput")
    dl3 = nc.dram_tensor("dl3", [P, C], f16, kind="ExternalInput")
    out = nc.dram_tensor("out", [NODES_PC, D], f32, kind="ExternalOutput")

    with tile.TileContext(nc) as tc:
        with (
            tc.tile_pool(name="meta", bufs=1) as meta,
            tc.tile_pool(name="sbuf_s", bufs=2) as sbuf_s,
            tc.tile_pool(name="oc", bufs=2) as oc,
            tc.tile_pool(name="small", bufs=4) as small,
            tc.tile_pool(name="psum", bufs=4, space="PSUM") as psum_tp,
        ):
            # ---------------- phase 0: metadata + mask ----------------
            et_t = meta.tile([P, C], f16)
            st_t = meta.tile([P, C], f16)
            dl3_t = meta.tile([P, C], f16)
            idx_t = meta.tile([P, NCHUNK * NBANKS * ICOLS], i16)
            # idx plane first: the gather calls depend ONLY on it, so
            # they can launch while the mask metadata still streams in
            nc.sync.dma_start(out=idx_t[:], in_=idx16[:])
            nc.scalar.dma_start(out=et_t[:], in_=etf[:])
            nc.scalar.dma_start(out=st_t[:], in_=stf[:])
            nc.scalar.dma_start(out=dl3_t[:], in_=dl3[:])

            # iota ramp 0..127 repeated K times: [P, K*P]
            iota_i = meta.tile([P, K * P], i32)
            nc.gpsimd.iota(iota_i[:], pattern=[[0, K], [1, P]], base=0,
                           channel_multiplier=0)
            iota_f = meta.tile([P, K * P], f16)
            nc.vector.tensor_copy(out=iota_f[:], in_=iota_i[:])

            # mask m = (st - et >= 0) & (st - et < TIME_WINDOW); all values
            # are small integers, exact in fp16.
            d_t = meta.tile([P, C], f16)
            nc.vector.tensor_tensor(out=d_t[:], in0=st_t[:], in1=et_t[:],
                                    op=OP.subtract)
            m1_t = meta.tile([P, C], f16)
            nc.vector.tensor_scalar(out=m1_t[:], in0=d_t[:], scalar1=0.0,
                                    scalar2=None, op0=OP.is_ge)
            m2_t = meta.tile([P, C], f16)
            nc.vector.tensor_scalar(out=m2_t[:], in0=d_t[:],
                                    scalar1=float(TIME_WINDOW),
                                    scalar2=None, op0=OP.is_lt)
            m_t = meta.tile([P, C], f16)
            nc.vector.tensor_tensor(out=m_t[:], in0=m1_t[:], in1=m2_t[:],
                                    op=OP.mult)
            # dl_eff = dl3 - 300*m   (in [0,128) iff mask==1)
            m300_t = meta.tile([P, C], f16)
            nc.vector.tensor_scalar(out=m300_t[:], in0=m_t[:], scalar1=300.0,
                                    scalar2=None, op0=OP.mult)
            dle_t = meta.tile([P, C], f16)
            nc.vector.tensor_tensor(out=dle_t[:], in0=dl3_t[:], in1=m300_t[:],
                                    op=OP.subtract)

            # Persistent triple-buffered gather target.  No zero-init is
            # needed: every slot (padding included) gathers a full valid
            # 512B row, so the matmul never reads unwritten bytes.
            g_bufs = []
            for i in range(3):
                g = meta.tile([P, NBANKS * CBLK * 2 * D], f16, tag=f"gbuf{i}")
                g_bufs.append(g)

            # ---------------- main loop ----------------
            for c in range(NCHUNK):
                g_t = g_bufs[c % 3]
                for j in range(NBANKS):
                    icol0 = (c * NBANKS + j) * ICOLS
                    nc.gpsimd.dma_gather(
                        out_ap=g_t[:]
                        .rearrange("p (k c) -> p k c", c=2 * D)[
                            :, j * CBLK : (j + 1) * CBLK, :
                        ],
                        in_ap=x16[j * BANK :, :],
                        idxs_ap=idx_t[:, icol0 : icol0 + ICOLS],
                        num_idxs=NIDX,
                        num_idxs_reg=NIDX,
                        elem_size=2 * D,
                        single_packet=False,
                        queue_num=j,
                    )

                # batched one-hot build for the whole chunk:
                # S[p, (wl k), m] = (iota[m] == dl_eff[p, w*K+k])
                s_t = sbuf_s.tile([P, CW * K * P], f16, tag="s")
                nc.vector.tensor_tensor(
                    out=s_t[:].rearrange("p (w k m) -> p w k m", k=K, m=P),
                    in0=iota_f[:]
                    .rearrange("p (k m) -> p k m", m=P)
                    .unsqueeze(1)
                    .to_broadcast([P, CW, K, P]),
                    in1=dle_t[:, c * CW * K : (c + 1) * CW * K]
                    .rearrange("p (w k) -> p w k", k=K)
                    .unsqueeze(3)
                    .to_broadcast([P, CW, K, P]),
                    op=OP.is_equal,
                )

                # x rows for the residual (contiguous: host-permuted order)
                x_t = oc.tile([P, CW * D], f32, tag="x")
                nc.scalar.dma_start(
                    out=x_t[:],
                    in_=xs[c * RPC : (c + 1) * RPC, :].rearrange(
                        "(p w) d -> p (w d)", p=P
                    ),
                )

                o_t = oc.tile([P, CW * D], f32, tag="o")
                for wl in range(CW):
                    ps = psum_tp.tile([P, D + 1], f32, tag="ps")
                    for k in range(K):
                        j, b = divmod(k, B)
                        gblk = j * CBLK + wl * B + b
                        nc.tensor.matmul(
                            out=ps[:],
                            lhsT=s_t[:, (wl * K + k) * P : (wl * K + k + 1) * P],
                            rhs=g_t[:, gblk * 2 * D : gblk * 2 * D + D + 1],
                            start=(k == 0),
                            stop=(k == K - 1),
                        )

                    cnt_t = small.tile([P, 1], f32, tag="cnt")
                    nc.vector.tensor_scalar(out=cnt_t[:], in0=ps[:, D : D + 1],
                                            scalar1=1.0, scalar2=None,
                                            op0=OP.max)
                    rcp_t = small.tile([P, 1], f32, tag="rcp")
                    nc.vector.reciprocal(out=rcp_t[:], in_=cnt_t[:])

                    osl = o_t[:, wl * D : (wl + 1) * D]
                    # mean = psum * (1/cnt) on ACT
                    nc.scalar.activation(
                        out=osl,
                        in_=ps[:, 0:D],
                        func=mybir.ActivationFunctionType.Copy,
                        scale=rcp_t[:, 0:1],
                    )
                    # out = mean + x on DVE
                    nc.vector.tensor_tensor(
                        out=osl, in0=osl, in1=x_t[:, wl * D : (wl + 1) * D],
                        op=OP.add,
                    )

                # store (contiguous: host-permuted row order)
                nc.sync.dma_start(
                    out=out[c * RPC : (c + 1) * RPC, :].rearrange(
                        "(p w) d -> p (w d)", p=P
                    ),
                    in_=o_t[:],
                )

    nc.compile()
    return nc


_PROGRAM_CACHE: dict[int, object] = {}


def _get_program(B: int):
    if B not in _PROGRAM_CACHE:
        _PROGRAM_CACHE[B] = build_program(B)
    return _PROGRAM_CACHE[B]


def _perm_rows(a, nchunk, cw):
    """[nchunk*CW*P, D] row permutation: (c, wl, p) -> (c, p, wl)."""
    return (
        a.reshape(nchunk, cw, P, -1).transpose(0, 2, 1, 3)
        .reshape(nchunk * cw * P, -1)
    )


def _unperm_rows(a, nchunk, cw):
    return (
        a.reshape(nchunk, P, cw, -1).transpose(0, 2, 1, 3)
        .reshape(nchunk * cw * P, -1)
    )


def _prep_inputs(x, edge_index, edge_time, seed_time):
    """Host-side layout: sort edges by (dst window, src bank) into the
    uniform slot grid; build metadata + wrapped int16 gather-index planes."""
    x = np.asarray(x, dtype=np.float32)
    ei = np.asarray(edge_index)
    et = np.asarray(edge_time).astype(np.int64)
    st = np.asarray(seed_time).astype(np.int64)
    N = x.shape[0]
    E = ei.shape[1]
    assert N <= NPAD and N <= XROWS

    src = ei[0].astype(np.int64)
    dst = ei[1].astype(np.int64)

    win = dst // P                      # global window id
    bank = src // BANK                  # 0..NBANKS-1
    gid = win * NBANKS + bank
    order = np.argsort(gid, kind="stable")
    gs = gid[order]
    binc = np.bincount(gid, minlength=NCORES * W * NBANKS)
    B = max(1, int(math.ceil(binc.max() / P)))
    K = NBANKS * B
    C = W * K

    offs = np.zeros(NCORES * W * NBANKS, dtype=np.int64)
    np.cumsum(binc[:-1], out=offs[1:])
    rank = np.arange(E, dtype=np.int64) - offs[gs]  # rank within (window, bank)
    win_s = gs // NBANKS
    bank_s = gs % NBANKS
    core_s = win_s // W
    wloc = win_s % W
    b = rank >> 7
    p = rank & (P - 1)

    # metadata slot grid: col = wloc*K + bank*B + b
    mcol = wloc * K + bank_s * B + b
    et_a = np.zeros((NCORES, P, C), dtype=np.float16)
    st_a = np.full((NCORES, P, C), -2000.0, dtype=np.float16)
    dl3_a = np.full((NCORES, P, C), 1300.0, dtype=np.float16)
    et_a[core_s, p, mcol] = et[order].astype(np.float16)
    st_a[core_s, p, mcol] = st[dst[order]].astype(np.float16)
    dl3_a[core_s, p, mcol] = (dst[order] % P).astype(np.float16) + 300.0

    # gather-index planes: per (chunk, bank) call, position
    # i = ((wl_in_chunk*B) + b)*128 + p, wrapped to [i%16, i//16],
    # replicated across the 8 16-partition groups.
    CBLK = CW * B
    NIDX = CBLK * P
    ICOLS = NIDX // 16
    chunk = wloc // CW
    wl = wloc % CW
    pos = (wl * B + b) * P + p
    icol = (chunk * NBANKS + bank_s) * ICOLS + pos // 16
    irow = pos % 16
    # padding slots must hit DISTINCT rows: a constant fill (row 0) funnels
    # every padding fetch onto one HBM row and serializes the DMA drain.
    ncols = NCHUNK * NBANKS * ICOLS
    spread = (np.arange(16 * ncols, dtype=np.int64) * 97) % BANK
    idx_a = np.broadcast_to(spread.reshape(16, ncols), (NCORES, 16, ncols))
    idx_a = np.ascontiguousarray(idx_a).astype(np.int16)
    idx_a[core_s, irow, icol] = (src[order] - bank_s * BANK).astype(np.int16)
    idx_rep = np.tile(idx_a, (1, 8, 1))

    x_pad = np.zeros((NPAD, D), dtype=np.float32)
    x_pad[:N] = x
    x16 = np.zeros((XROWS, 2 * D), dtype=np.float16)
    x16[:N, :D] = x.astype(np.float16)
    x16[:, D] = 1.0  # ones column -> count accumulation rides the matmul
    x_shards = x_pad.reshape(NCORES, NODES_PC, D)

    in_maps = [
        {
            "x16": x16,
            "xs": np.ascontiguousarray(_perm_rows(x_shards[c], NCHUNK, CW)),
            "idx16": idx_rep[c],
            "etf": et_a[c],
            "stf": st_a[c],
            "dl3": dl3_a[c],
        }
        for c in range(NCORES)
    ]
    return in_maps, B, N


def kernel(x, edge_index, edge_time, seed_time):
    in_maps, B, N = _prep_inputs(x, edge_index, edge_time, seed_time)
    nc = _get_program(B)
    res = run_bass_kernel_spmd(nc, in_maps, core_ids=list(range(NCORES)))
    out = np.concatenate(
        [_unperm_rows(res.results[c]["out"], NCHUNK, CW) for c in range(NCORES)],
        axis=0,
    )
    return np.ascontiguousarray(out[:N]).astype(np.float32)

